# revision 26
# baseline (speedup 1.0000x reference)
"""Bass/Trainium2 kernel for nn_EnhancedMultiHeadAttention (sparse_attention).

out[b,h,i,j] = softmax_j( (q_bh i . k_bh j) * sc + relbias[b,i,j] + mask_term[b,i,j] )
  q = query @ Wq.T + bq   (sc = 1/sqrt(64) folded into Wq/bq on host)
  relbias[b,i,j] = (mean_h q[b,h,i,:]) . rel_k_table[clip(j-i,-128,128)+128, :] * sc
  mask_term = 0 where mask==1 else -1e9

Sharding: 8 cores = 4 batches x 2 head-halves (8 heads per core).
Host pre-transposes per-batch activations (query[b].T) so the contraction dim
(D) lands on SBUF partitions; the head-mean projection is folded into a
[64,1024] weight since the projection is linear. Mask ships as int8.

Per-core device program (all matmuls in float32r; measured ~280ns per
N=512 matmul on TRN2):
  1. qT[512,S] projection + head-mean qmT[64,S] (PSUM accumulate over D,
     bias added during ACT evacuation).
  2. W = qm @ rel_k_table.T -> [S,257]; pad edges (clip regions) to [S,511];
     bounce through DRAM and read back with a skewed access pattern
     (partition step 510) to materialize the diagonal band bias[i, j-i+128].
     Runs before the k projection so the DRAM roundtrip overlaps PE work.
  3. combined[b] = (mask-1)*1e9 + relbias, assembled once per batch
     ([S,S] in SBUF), shared by all 8 heads.
  4. kT[512,S] projection.
  5. Main loop, row-tile major: scores PSUM = qk matmuls (K=64) +
     identity-matmul accumulate of combined; ACT exp with accum_out row
     sums; DVE reciprocal + per-partition normalize; DMA out.
"""

import numpy as np

B, S, D, H = 4, 1024, 1024, 16
DK = 64          # head dim
MAXREL = 128
NREL = 2 * MAXREL + 1          # 257
WPADW = 2 * MAXREL + NREL - 2  # 511 = 127 + 257 + 127
NRELP = 260     # rel matmul free dim padded for fp32r ISA restrictions
HPC = 8          # heads per core
DHALF = 512      # projected dims per core
NCORES = 8
PT = 128         # partition tile
NT = S // PT     # 8 row tiles

_CACHE = {}


def _build():
    from contextlib import ExitStack

    import concourse.bass as bass
    import concourse.mybir as mybir
    import concourse.tile as tile
    from concourse import bacc
    from concourse.tile import add_dep_helper

    F32 = mybir.dt.float32
    F16 = mybir.dt.float16
    I8 = mybir.dt.int8
    AF = mybir.ActivationFunctionType

    nc = bacc.Bacc("TRN2", target_bir_lowering=False, debug=False)

    xT = nc.dram_tensor("xT", [D, S], F16, kind="ExternalInput")
    kTx = nc.dram_tensor("kTx", [D, S], F16, kind="ExternalInput")
    maskb = nc.dram_tensor("maskb", [S, S], I8, kind="ExternalInput")
    wqT = nc.dram_tensor("wqT", [D, DHALF], F16, kind="ExternalInput")
    wkT = nc.dram_tensor("wkT", [D, DHALF], F16, kind="ExternalInput")
    bq4 = nc.dram_tensor("bq4", [PT, 4], F32, kind="ExternalInput")
    bk4 = nc.dram_tensor("bk4", [PT, 4], F32, kind="ExternalInput")
    wmT = nc.dram_tensor("wmT", [D, DK], F16, kind="ExternalInput")
    bm1 = nc.dram_tensor("bm1", [DK, 1], F32, kind="ExternalInput")
    tT = nc.dram_tensor("tT", [DK, NRELP], F16, kind="ExternalInput")
    out_d = nc.dram_tensor("out", [HPC, S, S], F32, kind="ExternalOutput")
    wpad_d = nc.dram_tensor("wpad_scratch", [S, WPADW], F16)
    ident_d = nc.inline_tensor(np.eye(PT, dtype=np.float16), "ident")

    MASKV = 30000.0  # fp16-safe large negative bias for masked entries

    with tile.TileContext(nc) as tc, ExitStack() as ctx:
        persist = ctx.enter_context(tc.tile_pool(name="persist", bufs=1))
        bpool = ctx.enter_context(tc.tile_pool(name="bpool", bufs=2))
        wppool = ctx.enter_context(tc.tile_pool(name="wppool", bufs=2))
        epool = ctx.enter_context(tc.tile_pool(name="epool", bufs=8))
        opool = ctx.enter_context(tc.tile_pool(name="opool", bufs=8))
        spool = ctx.enter_context(tc.tile_pool(name="spool", bufs=8))
        psum = ctx.enter_context(tc.tile_pool(name="psum", bufs=3, space="PSUM"))
        psump = ctx.enter_context(tc.tile_pool(name="psump", bufs=1, space="PSUM"))

        # ---- small constants ----
        id_sb = persist.tile([PT, PT], F16, tag="ident")
        nc.sync.dma_start(id_sb[:], ident_d[:])
        bq_sb = persist.tile([PT, 4], F32, tag="bq")
        nc.sync.dma_start(bq_sb[:], bq4[:])
        bk_sb = persist.tile([PT, 4], F32, tag="bk")
        nc.sync.dma_start(bk_sb[:], bk4[:])
        bm_sb = persist.tile([DK, 1], F32, tag="bm")
        nc.sync.dma_start(bm_sb[:], bm1[:])
        tT_sb = persist.tile([DK, NRELP], F16, tag="tT")
        nc.sync.dma_start(tT_sb[:], tT[:])
        ones_sb = persist.tile([PT, MAXREL - 1], F32, tag="ones")
        nc.vector.memset(ones_sb[:], 1.0)

        # ---- PE warmup: dense dummy matmuls while input DMAs land, so the
        # HAM clock-gate is at 8/8 when real work starts ----
        warm_sb = persist.tile([PT, DHALF], F16, tag="warm")
        nc.vector.memset(warm_sb[:], 0.0)
        wps = psump.tile([PT, DHALF], F32, tag="psp", name="warmps")
        for i in range(60):
            nc.tensor.matmul(wps[:], id_sb[:], warm_sb[:], start=True, stop=True)

        # ---- bulk input loads: one DMA per tensor (chunk-tiled into wide
        # SBUF tensors via 3D access patterns) ----
        def load_all(pool_, name, dram, rows, width, dt_):
            t = pool_.tile([PT, NT * width], dt_, tag=name, name=name)
            srcap = bass.AP(dram, 0, [[width, PT], [PT * width, NT], [1, width]])
            nc.sync.dma_start(t[:].rearrange("p (c s) -> p c s", s=width), srcap)
            return t

        x_all = load_all(persist, "x_all", xT, D, S, F16)
        wm_all = load_all(persist, "wm_all", wmT, D, DK, F16)
        wq_all = load_all(persist, "wq_all", wqT, D, DHALF, F16)
        x_tiles = [x_all[:, kc * S:(kc + 1) * S] for kc in range(NT)]
        wm_sb = [wm_all[:, kc * DK:(kc + 1) * DK] for kc in range(NT)]
        wq_tiles = [wq_all[:, kc * DHALF:(kc + 1) * DHALF] for kc in range(NT)]

        qmps = psump.tile([DK, S], F32, tag="psp", name="qmps")
        for kc in range(NT):
            for nh in range(2):
                nhs = slice(nh * DHALF, (nh + 1) * DHALF)
                nc.tensor.matmul(qmps[:, nhs], wm_sb[kc][:], x_tiles[kc][:, nhs],
                                 start=(kc == 0), stop=(kc == NT - 1))
        qmT_sb = persist.tile([DK, S], F16, tag="qmT")
        nc.vector.tensor_scalar_add(qmT_sb[:], qmps[:], bm_sb[:])

        wk_all = load_all(persist, "wk_all", wkT, D, DHALF, F16)
        k_all = load_all(persist, "k_all", kTx, D, S, F16)
        wk_tiles = [wk_all[:, kc * DHALF:(kc + 1) * DHALF] for kc in range(NT)]
        k_tiles = [k_all[:, kc * S:(kc + 1) * S] for kc in range(NT)]

        # ---- per head-pair: project qT[t], kT[t], then main loop over m ----
        qT_sb = [persist.tile([PT, S], F16, tag=f"qT{i}", name=f"qT{i}")
                 for i in range(4)]
        kT_sb = [persist.tile([PT, S], F16, tag=f"kT{i}", name=f"kT{i}")
                 for i in range(4)]

        def project(t, w_tiles, x_t, dst, bias_sb, pstag):
            ps = psump.tile([PT, S], F32, tag="psp", name=f"proj{pstag}{t}")
            for nh in range(2):
                nhs = slice(nh * DHALF, (nh + 1) * DHALF)
                for kc in range(NT):
                    nc.tensor.matmul(ps[:, nhs],
                                     w_tiles[kc][:, t * PT:(t + 1) * PT],
                                     x_t[kc][:, nhs],
                                     start=(kc == 0), stop=(kc == NT - 1))
            nc.vector.tensor_scalar_add(dst[:], ps[:], bias_sb[:, t:t + 1])

        # t=0 projections first: they give PE dense work that overlaps the
        # rel-bias DRAM roundtrip and comb assembly below
        project(0, wq_tiles, x_tiles, qT_sb[0], bq_sb, "psA")
        project(0, wk_tiles, k_tiles, kT_sb[0], bk_sb, "psB")

        # ---- rel bias: W = qm @ T.T, pad to fp16 [S,511], DRAM skew, comb ----
        band_info = [(max(0, PT * (m - 1)), min(S, PT * (m + 2)))
                     for m in range(NT)]
        mask_all = persist.tile([PT, NT * S], I8, tag="mask_all")
        nc.gpsimd.dma_start(
            mask_all[:].rearrange("p (c s) -> p c s", s=S),
            bass.AP(maskb, 0, [[S, PT], [PT * S, NT], [1, S]]))
        w0_sb = [persist.tile([PT, 1], F32, tag=f"w0_{m}", name=f"w0_{m}")
                 for m in range(NT)]
        w256_sb = [persist.tile([PT, 1], F32, tag=f"w256_{m}",
                                name=f"w256_{m}") for m in range(NT)]
        comb_sb = [persist.tile([PT, S], F16, tag=f"comb{m}", name=f"comb{m}")
                   for m in range(NT)]
        for m in range(NT):
            jlo, jhi = band_info[m]
            ps = psump.tile([PT, NRELP], F32, tag="psp", name=f"wps{m}")
            nc.tensor.matmul(ps[:], qmT_sb[:, m * PT:(m + 1) * PT], tT_sb[:],
                             start=True, stop=True)
            wp = wppool.tile([PT, WPADW], F16, tag="wpad", name=f"wpad{m}")
            nc.vector.tensor_copy(wp[:, MAXREL - 1:MAXREL - 1 + NREL],
                                  ps[:, 0:NREL])
            nc.vector.tensor_copy(w0_sb[m][:], ps[:, 0:1])
            nc.vector.tensor_copy(w256_sb[m][:], ps[:, NREL - 1:NREL])
            nc.vector.tensor_scalar_mul(wp[:, 0:MAXREL - 1], ones_sb[:],
                                        w0_sb[m][:])
            nc.vector.tensor_scalar_mul(wp[:, MAXREL - 1 + NREL:WPADW],
                                        ones_sb[:], w256_sb[m][:])
            wi = nc.gpsimd.dma_start(wpad_d[m * PT:(m + 1) * PT, :], wp[:])
            # skewed read: band[p, jj] = wpad[m*128+p, (jlo+jj)-(m*128+p)+255]
            bt = bpool.tile([PT, jhi - jlo], F16, tag="band", name=f"band{m}")
            srcap = bass.AP(wpad_d, PT * (WPADW - 1) * m + jlo + (WPADW // 2),
                            [[WPADW - 1, PT], [1, jhi - jlo]])
            ri = nc.gpsimd.dma_start(bt[:], srcap)
            add_dep_helper(ri.ins, wi.ins, reason="wpad DRAM RAW")
            # combined bias: (mask-1)*MASKV + relbias, fp16
            cb = comb_sb[m]
            nc.vector.tensor_scalar(cb[:], mask_all[:, m * S:(m + 1) * S], MASKV, -MASKV,
                                    mybir.AluOpType.mult, mybir.AluOpType.add)
            nc.vector.tensor_add(cb[:, jlo:jhi], cb[:, jlo:jhi], bt[:])
            if jlo > 0:
                nc.vector.tensor_scalar_add(cb[:, 0:jlo], cb[:, 0:jlo],
                                            w0_sb[m][:])
            if jhi < S:
                nc.vector.tensor_scalar_add(cb[:, jhi:S], cb[:, jhi:S],
                                            w256_sb[m][:])

        for t in range(4):
            if t > 0:
                project(t, wq_tiles, x_tiles, qT_sb[t], bq_sb, "psA")
                project(t, wk_tiles, k_tiles, kT_sb[t], bk_sb, "psB")
            for m in range(NT):
                mb = slice(m * PT, (m + 1) * PT)
                ps0 = psum.tile([PT, S], F32, tag="ps", name=f"ps0_{t}_{m}")
                ps1 = psum.tile([PT, S], F32, tag="ps", name=f"ps1_{t}_{m}")
                # one weight load per lhsT: headA both halves, headB both
                # halves, identity all four accumulate streams
                for nh in range(2):
                    nhs = slice(nh * DHALF, (nh + 1) * DHALF)
                    nc.tensor.matmul(ps0[:, nhs], qT_sb[t][0:DK, mb],
                                     kT_sb[t][0:DK, nhs], start=True,
                                     stop=False)
                for nh in range(2):
                    nhs = slice(nh * DHALF, (nh + 1) * DHALF)
                    nc.tensor.matmul(ps1[:, nhs], qT_sb[t][DK:PT, mb],
                                     kT_sb[t][DK:PT, nhs], start=True,
                                     stop=False)
                for psx in (ps0, ps1):
                    for nh in range(2):
                        nhs = slice(nh * DHALF, (nh + 1) * DHALF)
                        nc.tensor.matmul(psx[:, nhs], id_sb[:],
                                         comb_sb[m][:, nhs],
                                         start=False, stop=True)
                for hi, psx in ((0, ps0), (1, ps1)):
                    h = 2 * t + hi
                    e = epool.tile([PT, S], F32, tag="e", name=f"e{h}_{m}")
                    sm = spool.tile([PT, 1], F32, tag="s", name=f"s{h}_{m}")
                    nc.scalar.activation(e[:], psx[:], AF.Exp, bias=0.0,
                                         scale=1.0, accum_out=sm[:])
                    r = spool.tile([PT, 1], F32, tag="r", name=f"r{h}_{m}")
                    nc.vector.reciprocal(r[:], sm[:])
                    o = opool.tile([PT, S], F32, tag="o", name=f"o{h}_{m}")
                    nc.vector.tensor_scalar_mul(o[:], e[:], r[:])
                    nc.sync.dma_start(out_d[h, mb, :], o[:])

    nc.compile()
    return nc


def _get_nc():
    if "nc" not in _CACHE:
        _CACHE["nc"] = _build()
    return _CACHE["nc"]


def _prep_inputs(query, key, mask, Wq, bq, Wk, bk, rel_k_table):
    """Host-side sharding prep -> 8 per-core input dicts."""
    sc = 1.0 / np.sqrt(np.float32(DK))
    query = np.asarray(query, dtype=np.float32)
    key = np.asarray(key, dtype=np.float32)
    mask8 = np.ascontiguousarray(np.asarray(mask).astype(np.int8))
    Wq = np.asarray(Wq, dtype=np.float32)
    bq = np.asarray(bq, dtype=np.float32)
    Wk = np.asarray(Wk, dtype=np.float32)
    bk = np.asarray(bk, dtype=np.float32)
    T = np.asarray(rel_k_table, dtype=np.float32)

    WqTs = np.ascontiguousarray((Wq * sc).T)       # [D, D]
    WkT = np.ascontiguousarray(Wk.T)               # [D, D]
    bqs = bq * sc
    Wm16 = np.ascontiguousarray(((Wq.reshape(H, DK, D).mean(0) * sc).T).astype(np.float16))
    bm = (bq.reshape(H, DK).mean(0) * sc).reshape(DK, 1).astype(np.float32)
    tTc16 = np.zeros((DK, NRELP), np.float16)
    tTc16[:, :NREL] = T.T.astype(np.float16)       # [64, 260] zero-padded

    xT = [np.ascontiguousarray(query[b].T.astype(np.float16)) for b in range(B)]
    kT = [np.ascontiguousarray(key[b].T.astype(np.float16)) for b in range(B)]

    in_maps = []
    for c in range(NCORES):
        b, hh = divmod(c, 2)
        cols = slice(hh * DHALF, (hh + 1) * DHALF)
        in_maps.append(dict(
            xT=xT[b], kTx=kT[b], maskb=mask8[b],
            wqT=np.ascontiguousarray(WqTs[:, cols].astype(np.float16)),
            wkT=np.ascontiguousarray(WkT[:, cols].astype(np.float16)),
            bq4=np.ascontiguousarray(bqs[cols].reshape(4, PT).T),
            bk4=np.ascontiguousarray(bk[cols].reshape(4, PT).T),
            wmT=Wm16, bm1=bm, tT=tTc16,
        ))
    return in_maps


def run(inputs: dict, trace: bool = False):
    from concourse.bass_utils import run_bass_kernel_spmd

    nc = _get_nc()
    in_maps = _prep_inputs(**inputs)
    res = run_bass_kernel_spmd(nc, in_maps, core_ids=list(range(NCORES)),
                               trace=trace)
    out = np.empty((B, H, S, S), dtype=np.float32)
    for c in range(NCORES):
        b, hh = divmod(c, 2)
        out[b, hh * HPC:(hh + 1) * HPC] = res.results[c]["out"]
    return out, res


def kernel(**inputs) -> np.ndarray:
    out, _ = run(inputs)
    return out


# revision 27
# speedup vs baseline: 1.0074x; 1.0074x over previous
"""Bass/Trainium2 kernel for nn_EnhancedMultiHeadAttention (sparse_attention).

out[b,h,i,j] = softmax_j( (q_bh i . k_bh j) * sc + relbias[b,i,j] + mask_term[b,i,j] )
  q = query @ Wq.T + bq   (sc = 1/sqrt(64) folded into Wq/bq on host)
  relbias[b,i,j] = (mean_h q[b,h,i,:]) . rel_k_table[clip(j-i,-128,128)+128, :] * sc
  mask_term = 0 where mask==1 else -1e9

Sharding: 8 cores = 4 batches x 2 head-halves (8 heads per core).
Host pre-transposes per-batch activations (query[b].T) so the contraction dim
(D) lands on SBUF partitions; the head-mean projection is folded into a
[64,1024] weight since the projection is linear. Mask ships as int8.

Per-core device program (all matmuls in float32r; measured ~280ns per
N=512 matmul on TRN2):
  1. qT[512,S] projection + head-mean qmT[64,S] (PSUM accumulate over D,
     bias added during ACT evacuation).
  2. W = qm @ rel_k_table.T -> [S,257]; pad edges (clip regions) to [S,511];
     bounce through DRAM and read back with a skewed access pattern
     (partition step 510) to materialize the diagonal band bias[i, j-i+128].
     Runs before the k projection so the DRAM roundtrip overlaps PE work.
  3. combined[b] = (mask-1)*1e9 + relbias, assembled once per batch
     ([S,S] in SBUF), shared by all 8 heads.
  4. kT[512,S] projection.
  5. Main loop, row-tile major: scores PSUM = qk matmuls (K=64) +
     identity-matmul accumulate of combined; ACT exp with accum_out row
     sums; DVE reciprocal + per-partition normalize; DMA out.
"""

import numpy as np

B, S, D, H = 4, 1024, 1024, 16
DK = 64          # head dim
MAXREL = 128
NREL = 2 * MAXREL + 1          # 257
WPADW = 2 * MAXREL + NREL - 2  # 511 = 127 + 257 + 127
NRELP = 260     # rel matmul free dim padded for fp32r ISA restrictions
HPC = 8          # heads per core
DHALF = 512      # projected dims per core
NCORES = 8
PT = 128         # partition tile
NT = S // PT     # 8 row tiles

_CACHE = {}


def _build():
    from contextlib import ExitStack

    import concourse.bass as bass
    import concourse.mybir as mybir
    import concourse.tile as tile
    from concourse import bacc
    from concourse.tile import add_dep_helper

    F32 = mybir.dt.float32
    F16 = mybir.dt.float16
    I8 = mybir.dt.int8
    AF = mybir.ActivationFunctionType

    nc = bacc.Bacc("TRN2", target_bir_lowering=False, debug=False)

    xT = nc.dram_tensor("xT", [D, S], F16, kind="ExternalInput")
    kTx = nc.dram_tensor("kTx", [D, S], F16, kind="ExternalInput")
    maskb = nc.dram_tensor("maskb", [S, S], I8, kind="ExternalInput")
    wqT = nc.dram_tensor("wqT", [D, DHALF], F16, kind="ExternalInput")
    wkT = nc.dram_tensor("wkT", [D, DHALF], F16, kind="ExternalInput")
    bq4 = nc.dram_tensor("bq4", [PT, 4], F32, kind="ExternalInput")
    bk4 = nc.dram_tensor("bk4", [PT, 4], F32, kind="ExternalInput")
    wmT = nc.dram_tensor("wmT", [D, DK], F16, kind="ExternalInput")
    bm1 = nc.dram_tensor("bm1", [DK, 1], F32, kind="ExternalInput")
    tT = nc.dram_tensor("tT", [DK, NRELP], F16, kind="ExternalInput")
    out_d = nc.dram_tensor("out", [HPC, S, S], F32, kind="ExternalOutput")
    wpad_d = nc.dram_tensor("wpad_scratch", [S, WPADW], F16)
    ident_d = nc.inline_tensor(np.eye(PT, dtype=np.float16), "ident")

    MASKV = 30000.0  # fp16-safe large negative bias for masked entries

    with tile.TileContext(nc) as tc, ExitStack() as ctx:
        persist = ctx.enter_context(tc.tile_pool(name="persist", bufs=1))
        bpool = ctx.enter_context(tc.tile_pool(name="bpool", bufs=2))
        wppool = ctx.enter_context(tc.tile_pool(name="wppool", bufs=2))
        epool = ctx.enter_context(tc.tile_pool(name="epool", bufs=6))
        opool = ctx.enter_context(tc.tile_pool(name="opool", bufs=6))
        spool = ctx.enter_context(tc.tile_pool(name="spool", bufs=8))
        psum = ctx.enter_context(tc.tile_pool(name="psum", bufs=3, space="PSUM"))
        psump = ctx.enter_context(tc.tile_pool(name="psump", bufs=1, space="PSUM"))

        # ---- small constants ----
        id_sb = persist.tile([PT, PT], F16, tag="ident")
        nc.sync.dma_start(id_sb[:], ident_d[:])
        bq_sb = persist.tile([PT, 4], F32, tag="bq")
        nc.sync.dma_start(bq_sb[:], bq4[:])
        bk_sb = persist.tile([PT, 4], F32, tag="bk")
        nc.sync.dma_start(bk_sb[:], bk4[:])
        bm_sb = persist.tile([DK, 1], F32, tag="bm")
        nc.sync.dma_start(bm_sb[:], bm1[:])
        tT_sb = persist.tile([DK, NRELP], F16, tag="tT")
        nc.sync.dma_start(tT_sb[:], tT[:])
        ones_sb = persist.tile([PT, MAXREL - 1], F32, tag="ones")
        nc.vector.memset(ones_sb[:], 1.0)

        # ---- PE warmup: dense dummy matmuls while input DMAs land, so the
        # HAM clock-gate is at 8/8 when real work starts ----
        warm_sb = persist.tile([PT, DHALF], F16, tag="warm")
        nc.vector.memset(warm_sb[:], 0.0)
        wps = psump.tile([PT, DHALF], F32, tag="psp", name="warmps")
        for i in range(60):
            nc.tensor.matmul(wps[:], id_sb[:], warm_sb[:], start=True, stop=True)

        # ---- bulk input loads: one DMA per tensor (chunk-tiled into wide
        # SBUF tensors via 3D access patterns) ----
        def load_all(pool_, name, dram, rows, width, dt_):
            t = pool_.tile([PT, NT * width], dt_, tag=name, name=name)
            srcap = bass.AP(dram, 0, [[width, PT], [PT * width, NT], [1, width]])
            nc.sync.dma_start(t[:].rearrange("p (c s) -> p c s", s=width), srcap)
            return t

        x_all = load_all(persist, "x_all", xT, D, S, F16)
        wm_all = load_all(persist, "wm_all", wmT, D, DK, F16)
        wq_all = load_all(persist, "wq_all", wqT, D, DHALF, F16)
        x_tiles = [x_all[:, kc * S:(kc + 1) * S] for kc in range(NT)]
        wm_sb = [wm_all[:, kc * DK:(kc + 1) * DK] for kc in range(NT)]
        wq_tiles = [wq_all[:, kc * DHALF:(kc + 1) * DHALF] for kc in range(NT)]

        qmps = psump.tile([DK, S], F32, tag="psp", name="qmps")
        for kc in range(NT):
            for nh in range(2):
                nhs = slice(nh * DHALF, (nh + 1) * DHALF)
                nc.tensor.matmul(qmps[:, nhs], wm_sb[kc][:], x_tiles[kc][:, nhs],
                                 start=(kc == 0), stop=(kc == NT - 1))
        qmT_sb = persist.tile([DK, S], F16, tag="qmT")
        nc.vector.tensor_scalar_add(qmT_sb[:], qmps[:], bm_sb[:])

        wk_all = load_all(persist, "wk_all", wkT, D, DHALF, F16)
        k_all = load_all(persist, "k_all", kTx, D, S, F16)
        wk_tiles = [wk_all[:, kc * DHALF:(kc + 1) * DHALF] for kc in range(NT)]
        k_tiles = [k_all[:, kc * S:(kc + 1) * S] for kc in range(NT)]

        # ---- per head-pair: project qT[t], kT[t], then main loop over m ----
        qT_sb = [persist.tile([PT, S], F16, tag=f"qT{i}", name=f"qT{i}")
                 for i in range(4)]
        kT_sb = [persist.tile([PT, S], F16, tag=f"kT{i}", name=f"kT{i}")
                 for i in range(4)]

        def project(t, w_tiles, x_t, dst, bias_sb, pstag):
            ps = psump.tile([PT, S], F32, tag="psp", name=f"proj{pstag}{t}")
            for nh in range(2):
                nhs = slice(nh * DHALF, (nh + 1) * DHALF)
                for kc in range(NT):
                    nc.tensor.matmul(ps[:, nhs],
                                     w_tiles[kc][:, t * PT:(t + 1) * PT],
                                     x_t[kc][:, nhs],
                                     start=(kc == 0), stop=(kc == NT - 1))
            nc.vector.tensor_scalar_add(dst[:], ps[:], bias_sb[:, t:t + 1])

        # t=0 projections first: they give PE dense work that overlaps the
        # rel-bias DRAM roundtrip and comb assembly below
        project(0, wq_tiles, x_tiles, qT_sb[0], bq_sb, "psA")
        project(0, wk_tiles, k_tiles, kT_sb[0], bk_sb, "psB")

        # ---- rel bias: W = qm @ T.T, pad to fp16 [S,511], DRAM skew, comb ----
        band_info = [(max(0, PT * (m - 1)), min(S, PT * (m + 2)))
                     for m in range(NT)]
        mask_all = persist.tile([PT, NT * S], I8, tag="mask_all")
        nc.gpsimd.dma_start(
            mask_all[:].rearrange("p (c s) -> p c s", s=S),
            bass.AP(maskb, 0, [[S, PT], [PT * S, NT], [1, S]]))
        w0_sb = [persist.tile([PT, 1], F32, tag=f"w0_{m}", name=f"w0_{m}")
                 for m in range(NT)]
        w256_sb = [persist.tile([PT, 1], F32, tag=f"w256_{m}",
                                name=f"w256_{m}") for m in range(NT)]
        comb_sb = [persist.tile([PT, S], F16, tag=f"comb{m}", name=f"comb{m}")
                   for m in range(NT)]
        for m in range(NT):
            jlo, jhi = band_info[m]
            ps = psump.tile([PT, NRELP], F32, tag="psp", name=f"wps{m}")
            nc.tensor.matmul(ps[:], qmT_sb[:, m * PT:(m + 1) * PT], tT_sb[:],
                             start=True, stop=True)
            wp = wppool.tile([PT, WPADW], F16, tag="wpad", name=f"wpad{m}")
            nc.vector.tensor_copy(wp[:, MAXREL - 1:MAXREL - 1 + NREL],
                                  ps[:, 0:NREL])
            nc.vector.tensor_copy(w0_sb[m][:], ps[:, 0:1])
            nc.vector.tensor_copy(w256_sb[m][:], ps[:, NREL - 1:NREL])
            nc.vector.tensor_scalar_mul(wp[:, 0:MAXREL - 1], ones_sb[:],
                                        w0_sb[m][:])
            nc.vector.tensor_scalar_mul(wp[:, MAXREL - 1 + NREL:WPADW],
                                        ones_sb[:], w256_sb[m][:])
            wi = nc.gpsimd.dma_start(wpad_d[m * PT:(m + 1) * PT, :], wp[:])
            # skewed read: band[p, jj] = wpad[m*128+p, (jlo+jj)-(m*128+p)+255]
            bt = bpool.tile([PT, jhi - jlo], F16, tag="band", name=f"band{m}")
            srcap = bass.AP(wpad_d, PT * (WPADW - 1) * m + jlo + (WPADW // 2),
                            [[WPADW - 1, PT], [1, jhi - jlo]])
            ri = nc.gpsimd.dma_start(bt[:], srcap)
            add_dep_helper(ri.ins, wi.ins, reason="wpad DRAM RAW")
            # combined bias: (mask-1)*MASKV + relbias, fp16
            cb = comb_sb[m]
            nc.vector.tensor_scalar(cb[:], mask_all[:, m * S:(m + 1) * S], MASKV, -MASKV,
                                    mybir.AluOpType.mult, mybir.AluOpType.add)
            nc.vector.tensor_add(cb[:, jlo:jhi], cb[:, jlo:jhi], bt[:])
            if jlo > 0:
                nc.vector.tensor_scalar_add(cb[:, 0:jlo], cb[:, 0:jlo],
                                            w0_sb[m][:])
            if jhi < S:
                nc.vector.tensor_scalar_add(cb[:, jhi:S], cb[:, jhi:S],
                                            w256_sb[m][:])

        for t in range(4):
            if t > 0:
                project(t, wq_tiles, x_tiles, qT_sb[t], bq_sb, "psA")
                project(t, wk_tiles, k_tiles, kT_sb[t], bk_sb, "psB")
            for m in range(NT):
                mb = slice(m * PT, (m + 1) * PT)
                ps0 = psum.tile([PT, S], F32, tag="ps", name=f"ps0_{t}_{m}")
                ps1 = psum.tile([PT, S], F32, tag="ps", name=f"ps1_{t}_{m}")
                # one weight load per lhsT: headA both halves, headB both
                # halves, identity all four accumulate streams
                for nh in range(2):
                    nhs = slice(nh * DHALF, (nh + 1) * DHALF)
                    nc.tensor.matmul(ps0[:, nhs], qT_sb[t][0:DK, mb],
                                     kT_sb[t][0:DK, nhs], start=True,
                                     stop=False)
                for nh in range(2):
                    nhs = slice(nh * DHALF, (nh + 1) * DHALF)
                    nc.tensor.matmul(ps1[:, nhs], qT_sb[t][DK:PT, mb],
                                     kT_sb[t][DK:PT, nhs], start=True,
                                     stop=False)
                for psx in (ps0, ps1):
                    for nh in range(2):
                        nhs = slice(nh * DHALF, (nh + 1) * DHALF)
                        nc.tensor.matmul(psx[:, nhs], id_sb[:],
                                         comb_sb[m][:, nhs],
                                         start=False, stop=True)
                for hi, psx in ((0, ps0), (1, ps1)):
                    h = 2 * t + hi
                    e = epool.tile([PT, S], F32, tag="e", name=f"e{h}_{m}")
                    sm = spool.tile([PT, 1], F32, tag="s", name=f"s{h}_{m}")
                    nc.scalar.activation(e[:], psx[:], AF.Exp, bias=0.0,
                                         scale=1.0, accum_out=sm[:])
                    r = spool.tile([PT, 1], F32, tag="r", name=f"r{h}_{m}")
                    nc.vector.reciprocal(r[:], sm[:])
                    o = opool.tile([PT, S], F32, tag="o", name=f"o{h}_{m}")
                    nc.vector.tensor_scalar_mul(o[:], e[:], r[:])
                    nc.sync.dma_start(out_d[h, mb, :], o[:])

    nc.compile()
    return nc


def _get_nc():
    if "nc" not in _CACHE:
        _CACHE["nc"] = _build()
    return _CACHE["nc"]


def _prep_inputs(query, key, mask, Wq, bq, Wk, bk, rel_k_table):
    """Host-side sharding prep -> 8 per-core input dicts."""
    sc = 1.0 / np.sqrt(np.float32(DK))
    query = np.asarray(query, dtype=np.float32)
    key = np.asarray(key, dtype=np.float32)
    mask8 = np.ascontiguousarray(np.asarray(mask).astype(np.int8))
    Wq = np.asarray(Wq, dtype=np.float32)
    bq = np.asarray(bq, dtype=np.float32)
    Wk = np.asarray(Wk, dtype=np.float32)
    bk = np.asarray(bk, dtype=np.float32)
    T = np.asarray(rel_k_table, dtype=np.float32)

    WqTs = np.ascontiguousarray((Wq * sc).T)       # [D, D]
    WkT = np.ascontiguousarray(Wk.T)               # [D, D]
    bqs = bq * sc
    Wm16 = np.ascontiguousarray(((Wq.reshape(H, DK, D).mean(0) * sc).T).astype(np.float16))
    bm = (bq.reshape(H, DK).mean(0) * sc).reshape(DK, 1).astype(np.float32)
    tTc16 = np.zeros((DK, NRELP), np.float16)
    tTc16[:, :NREL] = T.T.astype(np.float16)       # [64, 260] zero-padded

    xT = [np.ascontiguousarray(query[b].T.astype(np.float16)) for b in range(B)]
    kT = [np.ascontiguousarray(key[b].T.astype(np.float16)) for b in range(B)]

    in_maps = []
    for c in range(NCORES):
        b, hh = divmod(c, 2)
        cols = slice(hh * DHALF, (hh + 1) * DHALF)
        in_maps.append(dict(
            xT=xT[b], kTx=kT[b], maskb=mask8[b],
            wqT=np.ascontiguousarray(WqTs[:, cols].astype(np.float16)),
            wkT=np.ascontiguousarray(WkT[:, cols].astype(np.float16)),
            bq4=np.ascontiguousarray(bqs[cols].reshape(4, PT).T),
            bk4=np.ascontiguousarray(bk[cols].reshape(4, PT).T),
            wmT=Wm16, bm1=bm, tT=tTc16,
        ))
    return in_maps


def run(inputs: dict, trace: bool = False):
    from concourse.bass_utils import run_bass_kernel_spmd

    nc = _get_nc()
    in_maps = _prep_inputs(**inputs)
    res = run_bass_kernel_spmd(nc, in_maps, core_ids=list(range(NCORES)),
                               trace=trace)
    out = np.empty((B, H, S, S), dtype=np.float32)
    for c in range(NCORES):
        b, hh = divmod(c, 2)
        out[b, hh * HPC:(hh + 1) * HPC] = res.results[c]["out"]
    return out, res


def kernel(**inputs) -> np.ndarray:
    out, _ = run(inputs)
    return out


# revision 28
# speedup vs baseline: 1.0449x; 1.0372x over previous
"""Bass/Trainium2 kernel for nn_EnhancedMultiHeadAttention (sparse_attention).

out[b,h,i,j] = softmax_j( (q_bh i . k_bh j) * sc + relbias[b,i,j] + mask_term[b,i,j] )
  q = query @ Wq.T + bq   (sc = 1/sqrt(64) folded into Wq/bq on host)
  relbias[b,i,j] = (mean_h q[b,h,i,:]) . rel_k_table[clip(j-i,-128,128)+128, :] * sc
  mask_term = 0 where mask==1 else -1e9

Sharding: 8 cores = 4 batches x 2 head-halves (8 heads per core).
Host pre-transposes per-batch activations (query[b].T) so the contraction dim
(D) lands on SBUF partitions; the head-mean projection is folded into a
[64,1024] weight since the projection is linear. Mask ships as int8.

Per-core device program (all matmuls in float32r; measured ~280ns per
N=512 matmul on TRN2):
  1. qT[512,S] projection + head-mean qmT[64,S] (PSUM accumulate over D,
     bias added during ACT evacuation).
  2. W = qm @ rel_k_table.T -> [S,257]; pad edges (clip regions) to [S,511];
     bounce through DRAM and read back with a skewed access pattern
     (partition step 510) to materialize the diagonal band bias[i, j-i+128].
     Runs before the k projection so the DRAM roundtrip overlaps PE work.
  3. combined[b] = (mask-1)*1e9 + relbias, assembled once per batch
     ([S,S] in SBUF), shared by all 8 heads.
  4. kT[512,S] projection.
  5. Main loop, row-tile major: scores PSUM = qk matmuls (K=64) +
     identity-matmul accumulate of combined; ACT exp with accum_out row
     sums; DVE reciprocal + per-partition normalize; DMA out.
"""

import numpy as np

B, S, D, H = 4, 1024, 1024, 16
DK = 64          # head dim
MAXREL = 128
NREL = 2 * MAXREL + 1          # 257
WPADW = 2 * MAXREL + NREL - 2  # 511 = 127 + 257 + 127
NRELP = 260     # rel matmul free dim padded for fp32r ISA restrictions
HPC = 8          # heads per core
DHALF = 512      # projected dims per core
NCORES = 8
PT = 128         # partition tile
NT = S // PT     # 8 row tiles

_CACHE = {}


def _build():
    from contextlib import ExitStack

    import concourse.bass as bass
    import concourse.mybir as mybir
    import concourse.tile as tile
    from concourse import bacc
    from concourse.tile import add_dep_helper

    F32 = mybir.dt.float32
    F16 = mybir.dt.float16
    I8 = mybir.dt.int8
    AF = mybir.ActivationFunctionType

    nc = bacc.Bacc("TRN2", target_bir_lowering=False, debug=False)

    xT = nc.dram_tensor("xT", [D, S], F16, kind="ExternalInput")
    kTx = nc.dram_tensor("kTx", [D, S], F16, kind="ExternalInput")
    maskb = nc.dram_tensor("maskb", [S, S], I8, kind="ExternalInput")
    wqT = nc.dram_tensor("wqT", [D, DHALF], F16, kind="ExternalInput")
    wkT = nc.dram_tensor("wkT", [D, DHALF], F16, kind="ExternalInput")
    bq4 = nc.dram_tensor("bq4", [PT, 4], F32, kind="ExternalInput")
    bk4 = nc.dram_tensor("bk4", [PT, 4], F32, kind="ExternalInput")
    wmT = nc.dram_tensor("wmT", [D, DK], F16, kind="ExternalInput")
    bm1 = nc.dram_tensor("bm1", [DK, 1], F32, kind="ExternalInput")
    tT = nc.dram_tensor("tT", [DK, NRELP], F16, kind="ExternalInput")
    out_d = nc.dram_tensor("out", [HPC, S, S], F32, kind="ExternalOutput")
    wpad_d = nc.dram_tensor("wpad_scratch", [S, WPADW], F16)
    ident_d = nc.inline_tensor(np.eye(PT, dtype=np.float16), "ident")

    MASKV = 30000.0  # fp16-safe large negative bias for masked entries

    with tile.TileContext(nc) as tc, ExitStack() as ctx:
        persist = ctx.enter_context(tc.tile_pool(name="persist", bufs=1))
        bpool = ctx.enter_context(tc.tile_pool(name="bpool", bufs=2))
        wppool = ctx.enter_context(tc.tile_pool(name="wppool", bufs=2))
        epool = ctx.enter_context(tc.tile_pool(name="epool", bufs=6))
        opool = ctx.enter_context(tc.tile_pool(name="opool", bufs=6))
        spool = ctx.enter_context(tc.tile_pool(name="spool", bufs=8))
        psum = ctx.enter_context(tc.tile_pool(name="psum", bufs=3, space="PSUM"))
        psump = ctx.enter_context(tc.tile_pool(name="psump", bufs=1, space="PSUM"))

        # ---- small constants ----
        id_sb = persist.tile([PT, PT], F16, tag="ident")
        nc.sync.dma_start(id_sb[:], ident_d[:])
        bq_sb = persist.tile([PT, 4], F32, tag="bq")
        nc.sync.dma_start(bq_sb[:], bq4[:])
        bk_sb = persist.tile([PT, 4], F32, tag="bk")
        nc.sync.dma_start(bk_sb[:], bk4[:])
        bm_sb = persist.tile([DK, 1], F32, tag="bm")
        nc.sync.dma_start(bm_sb[:], bm1[:])
        tT_sb = persist.tile([DK, NRELP], F16, tag="tT")
        nc.sync.dma_start(tT_sb[:], tT[:])
        ones_sb = persist.tile([PT, MAXREL - 1], F32, tag="ones")
        nc.vector.memset(ones_sb[:], 1.0)

        # ---- PE warmup: dense dummy matmuls while input DMAs land, so the
        # HAM clock-gate is at 8/8 when real work starts ----
        warm_sb = persist.tile([PT, DHALF], F16, tag="warm")
        nc.vector.memset(warm_sb[:], 0.0)
        wps = psump.tile([PT, DHALF], F32, tag="psp", name="warmps")
        for i in range(26):
            nc.tensor.matmul(wps[:], id_sb[:], warm_sb[:], start=True, stop=True)

        # ---- bulk input loads: one DMA per tensor (chunk-tiled into wide
        # SBUF tensors via 3D access patterns) ----
        def load_all(pool_, name, dram, rows, width, dt_, parts=1):
            t = pool_.tile([PT, NT * width], dt_, tag=name, name=name)
            cpp = NT // parts
            for pi in range(parts):
                srcap = bass.AP(dram, pi * cpp * PT * width,
                                [[width, PT], [PT * width, cpp], [1, width]])
                nc.sync.dma_start(
                    t[:, pi * cpp * width:(pi + 1) * cpp * width]
                    .rearrange("p (c s) -> p c s", s=width), srcap)
            return t

        x_all = load_all(persist, "x_all", xT, D, S, F16, parts=4)
        wm_all = load_all(persist, "wm_all", wmT, D, DK, F16)
        wq_all = load_all(persist, "wq_all", wqT, D, DHALF, F16, parts=2)
        x_tiles = [x_all[:, kc * S:(kc + 1) * S] for kc in range(NT)]
        wm_sb = [wm_all[:, kc * DK:(kc + 1) * DK] for kc in range(NT)]
        wq_tiles = [wq_all[:, kc * DHALF:(kc + 1) * DHALF] for kc in range(NT)]

        qmps = psump.tile([DK, S], F32, tag="psp", name="qmps")
        for kc in range(NT):
            for nh in range(2):
                nhs = slice(nh * DHALF, (nh + 1) * DHALF)
                nc.tensor.matmul(qmps[:, nhs], wm_sb[kc][:], x_tiles[kc][:, nhs],
                                 start=(kc == 0), stop=(kc == NT - 1))
        qmT_sb = persist.tile([DK, S], F16, tag="qmT")
        nc.vector.tensor_scalar_add(qmT_sb[:], qmps[:], bm_sb[:])

        wk_all = load_all(persist, "wk_all", wkT, D, DHALF, F16)
        k_all = load_all(persist, "k_all", kTx, D, S, F16, parts=2)
        wk_tiles = [wk_all[:, kc * DHALF:(kc + 1) * DHALF] for kc in range(NT)]
        k_tiles = [k_all[:, kc * S:(kc + 1) * S] for kc in range(NT)]

        # ---- per head-pair: project qT[t], kT[t], then main loop over m ----
        qT_sb = [persist.tile([PT, S], F16, tag=f"qT{i}", name=f"qT{i}")
                 for i in range(4)]
        kT_sb = [persist.tile([PT, S], F16, tag=f"kT{i}", name=f"kT{i}")
                 for i in range(4)]

        def project(t, w_tiles, x_t, dst, bias_sb, pstag):
            ps = psump.tile([PT, S], F32, tag="psp", name=f"proj{pstag}{t}")
            for nh in range(2):
                nhs = slice(nh * DHALF, (nh + 1) * DHALF)
                for kc in range(NT):
                    nc.tensor.matmul(ps[:, nhs],
                                     w_tiles[kc][:, t * PT:(t + 1) * PT],
                                     x_t[kc][:, nhs],
                                     start=(kc == 0), stop=(kc == NT - 1))
            nc.vector.tensor_scalar_add(dst[:], ps[:], bias_sb[:, t:t + 1])

        # t=0 projections first: they give PE dense work that overlaps the
        # rel-bias DRAM roundtrip and comb assembly below
        project(0, wq_tiles, x_tiles, qT_sb[0], bq_sb, "psA")
        project(0, wk_tiles, k_tiles, kT_sb[0], bk_sb, "psB")

        # ---- rel bias: W = qm @ T.T, pad to fp16 [S,511], DRAM skew, comb ----
        band_info = [(max(0, PT * (m - 1)), min(S, PT * (m + 2)))
                     for m in range(NT)]
        mask_all = persist.tile([PT, NT * S], I8, tag="mask_all")
        nc.gpsimd.dma_start(
            mask_all[:].rearrange("p (c s) -> p c s", s=S),
            bass.AP(maskb, 0, [[S, PT], [PT * S, NT], [1, S]]))
        w0_sb = [persist.tile([PT, 1], F32, tag=f"w0_{m}", name=f"w0_{m}")
                 for m in range(NT)]
        w256_sb = [persist.tile([PT, 1], F32, tag=f"w256_{m}",
                                name=f"w256_{m}") for m in range(NT)]
        comb_sb = [persist.tile([PT, S], F16, tag=f"comb{m}", name=f"comb{m}")
                   for m in range(NT)]
        for m in range(NT):
            jlo, jhi = band_info[m]
            ps = psump.tile([PT, NRELP], F32, tag="psp", name=f"wps{m}")
            nc.tensor.matmul(ps[:], qmT_sb[:, m * PT:(m + 1) * PT], tT_sb[:],
                             start=True, stop=True)
            wp = wppool.tile([PT, WPADW], F16, tag="wpad", name=f"wpad{m}")
            nc.vector.tensor_copy(wp[:, MAXREL - 1:MAXREL - 1 + NREL],
                                  ps[:, 0:NREL])
            nc.vector.tensor_copy(w0_sb[m][:], ps[:, 0:1])
            nc.vector.tensor_copy(w256_sb[m][:], ps[:, NREL - 1:NREL])
            nc.vector.tensor_scalar_mul(wp[:, 0:MAXREL - 1], ones_sb[:],
                                        w0_sb[m][:])
            nc.vector.tensor_scalar_mul(wp[:, MAXREL - 1 + NREL:WPADW],
                                        ones_sb[:], w256_sb[m][:])
            wi = nc.gpsimd.dma_start(wpad_d[m * PT:(m + 1) * PT, :], wp[:])
            # skewed read: band[p, jj] = wpad[m*128+p, (jlo+jj)-(m*128+p)+255]
            bt = bpool.tile([PT, jhi - jlo], F16, tag="band", name=f"band{m}")
            srcap = bass.AP(wpad_d, PT * (WPADW - 1) * m + jlo + (WPADW // 2),
                            [[WPADW - 1, PT], [1, jhi - jlo]])
            ri = nc.gpsimd.dma_start(bt[:], srcap)
            add_dep_helper(ri.ins, wi.ins, reason="wpad DRAM RAW")
            # combined bias: (mask-1)*MASKV + relbias, fp16
            cb = comb_sb[m]
            nc.vector.tensor_scalar(cb[:], mask_all[:, m * S:(m + 1) * S], MASKV, -MASKV,
                                    mybir.AluOpType.mult, mybir.AluOpType.add)
            nc.vector.tensor_add(cb[:, jlo:jhi], cb[:, jlo:jhi], bt[:])
            if jlo > 0:
                nc.vector.tensor_scalar_add(cb[:, 0:jlo], cb[:, 0:jlo],
                                            w0_sb[m][:])
            if jhi < S:
                nc.vector.tensor_scalar_add(cb[:, jhi:S], cb[:, jhi:S],
                                            w256_sb[m][:])

        for t in range(4):
            if t > 0:
                project(t, wq_tiles, x_tiles, qT_sb[t], bq_sb, "psA")
                project(t, wk_tiles, k_tiles, kT_sb[t], bk_sb, "psB")
            for m in range(NT):
                mb = slice(m * PT, (m + 1) * PT)
                ps0 = psum.tile([PT, S], F32, tag="ps", name=f"ps0_{t}_{m}")
                ps1 = psum.tile([PT, S], F32, tag="ps", name=f"ps1_{t}_{m}")
                # one weight load per lhsT: headA both halves, headB both
                # halves, identity all four accumulate streams
                for nh in range(2):
                    nhs = slice(nh * DHALF, (nh + 1) * DHALF)
                    nc.tensor.matmul(ps0[:, nhs], qT_sb[t][0:DK, mb],
                                     kT_sb[t][0:DK, nhs], start=True,
                                     stop=False)
                for nh in range(2):
                    nhs = slice(nh * DHALF, (nh + 1) * DHALF)
                    nc.tensor.matmul(ps1[:, nhs], qT_sb[t][DK:PT, mb],
                                     kT_sb[t][DK:PT, nhs], start=True,
                                     stop=False)
                for psx in (ps0, ps1):
                    for nh in range(2):
                        nhs = slice(nh * DHALF, (nh + 1) * DHALF)
                        nc.tensor.matmul(psx[:, nhs], id_sb[:],
                                         comb_sb[m][:, nhs],
                                         start=False, stop=True)
                for hi, psx in ((0, ps0), (1, ps1)):
                    h = 2 * t + hi
                    e = epool.tile([PT, S], F32, tag="e", name=f"e{h}_{m}")
                    sm = spool.tile([PT, 1], F32, tag="s", name=f"s{h}_{m}")
                    nc.scalar.activation(e[:], psx[:], AF.Exp, bias=0.0,
                                         scale=1.0, accum_out=sm[:])
                    r = spool.tile([PT, 1], F32, tag="r", name=f"r{h}_{m}")
                    nc.vector.reciprocal(r[:], sm[:])
                    o = opool.tile([PT, S], F32, tag="o", name=f"o{h}_{m}")
                    nc.vector.tensor_scalar_mul(o[:], e[:], r[:])
                    nc.sync.dma_start(out_d[h, mb, :], o[:])

    nc.compile()
    return nc


def _get_nc():
    if "nc" not in _CACHE:
        _CACHE["nc"] = _build()
    return _CACHE["nc"]


def _prep_inputs(query, key, mask, Wq, bq, Wk, bk, rel_k_table):
    """Host-side sharding prep -> 8 per-core input dicts."""
    sc = 1.0 / np.sqrt(np.float32(DK))
    query = np.asarray(query, dtype=np.float32)
    key = np.asarray(key, dtype=np.float32)
    mask8 = np.ascontiguousarray(np.asarray(mask).astype(np.int8))
    Wq = np.asarray(Wq, dtype=np.float32)
    bq = np.asarray(bq, dtype=np.float32)
    Wk = np.asarray(Wk, dtype=np.float32)
    bk = np.asarray(bk, dtype=np.float32)
    T = np.asarray(rel_k_table, dtype=np.float32)

    WqTs = np.ascontiguousarray((Wq * sc).T)       # [D, D]
    WkT = np.ascontiguousarray(Wk.T)               # [D, D]
    bqs = bq * sc
    Wm16 = np.ascontiguousarray(((Wq.reshape(H, DK, D).mean(0) * sc).T).astype(np.float16))
    bm = (bq.reshape(H, DK).mean(0) * sc).reshape(DK, 1).astype(np.float32)
    tTc16 = np.zeros((DK, NRELP), np.float16)
    tTc16[:, :NREL] = T.T.astype(np.float16)       # [64, 260] zero-padded

    xT = [np.ascontiguousarray(query[b].T.astype(np.float16)) for b in range(B)]
    kT = [np.ascontiguousarray(key[b].T.astype(np.float16)) for b in range(B)]

    in_maps = []
    for c in range(NCORES):
        b, hh = divmod(c, 2)
        cols = slice(hh * DHALF, (hh + 1) * DHALF)
        in_maps.append(dict(
            xT=xT[b], kTx=kT[b], maskb=mask8[b],
            wqT=np.ascontiguousarray(WqTs[:, cols].astype(np.float16)),
            wkT=np.ascontiguousarray(WkT[:, cols].astype(np.float16)),
            bq4=np.ascontiguousarray(bqs[cols].reshape(4, PT).T),
            bk4=np.ascontiguousarray(bk[cols].reshape(4, PT).T),
            wmT=Wm16, bm1=bm, tT=tTc16,
        ))
    return in_maps


def run(inputs: dict, trace: bool = False):
    from concourse.bass_utils import run_bass_kernel_spmd

    nc = _get_nc()
    in_maps = _prep_inputs(**inputs)
    res = run_bass_kernel_spmd(nc, in_maps, core_ids=list(range(NCORES)),
                               trace=trace)
    out = np.empty((B, H, S, S), dtype=np.float32)
    for c in range(NCORES):
        b, hh = divmod(c, 2)
        out[b, hh * HPC:(hh + 1) * HPC] = res.results[c]["out"]
    return out, res


def kernel(**inputs) -> np.ndarray:
    out, _ = run(inputs)
    return out
